# revision 1
# baseline (speedup 1.0000x reference)
"""Trainium2 Bass kernel: single-head causal attention with low-rank V.

Math (per batch b):
  Q = i@q, K = i@k                  [4096, 64]
  S = Q K^T  (causal mask, /8)      [4096, 4096]
  A = softmax(S)
  out = A @ ((i@v1) @ v2) = ((A @ (i@v1)) / l) @ v2   (low-rank reassociation)

Sharding: 8 cores = 4 batches x 2 halves. Core half h owns query tiles
g = 2t+h (t=0..15). One SPMD program; per-core differences are data only
(column-permuted i^T so own tiles sit at even block positions, plus two
[128,128] mask blocks and a [128,2] window tensor).

v3 design (all matmuls fp32r at >=256 free / f16):
  P1: packed QK^T projection (wq|wk), pv = (i@v1) via pvT + PE transpose
  P2 (pass A, row-major): S chunks -> row max m via DVE tensor_mask_reduce
      (causal windows are data), -m written into row 64 of Q^T
  P3 (pass B, transposed): S'^T[k,q] = [K^T;1]^T-style 65-row contraction
      = K^T q - m  (m folded into matmul), exp on ACT -> E^T f16 in SBUF
      directly the lhsT of O accumulation (no PE transposes of E, no DVE
      copy-backs), O[q,65] += E^T^T @ [pv|1] gives both O and l.
      Finalize: O/l -> f16 -> transpose -> @ v2 -> out.
"""

import sys

if "/opt/trn_rl_repo" not in sys.path:
    sys.path.insert(0, "/opt/trn_rl_repo")

from contextlib import ExitStack

import numpy as np

import concourse.bass as bass
import concourse.tile as tile
from concourse import bacc
from concourse import mybir
from concourse.bass_utils import run_bass_kernel_spmd

B, LN, IDM, HDM = 4, 4096, 512, 64
P = 128          # partitions / tile rows
NT = 16          # query tiles per core
NKT = 32         # key (pos-)tiles per batch
SC = 512         # key chunk (1 psum bank of fp32)
MASK_NEG = -60000.0  # "-inf" in f16-representable units (pre-scale)


def build_nc_v3():
    """v4: fp32 projections (exact Q,K); scores via compensated bf16x2
    matmuls (m1 = [Khi;1]^T[Qhi;-m], m2 = [Klo;Khi]^T[Qhi;Qlo]); pass A
    (row max) in plain bf16-hi; pv path in fp32r via bitcast."""
    f32 = mybir.dt.float32
    f16 = mybir.dt.float16
    bf16 = mybir.dt.bfloat16
    f32r = mybir.dt.float32r
    nc = bacc.Bacc()

    ih = nc.dram_tensor("ih", [IDM, LN], f16, kind="ExternalInput")
    il = nc.dram_tensor("il", [IDM, LN], f16, kind="ExternalInput")
    wh = nc.dram_tensor("wh", [IDM, P], f16, kind="ExternalInput")
    wl = nc.dram_tensor("wl", [IDM, P], f16, kind="ExternalInput")
    v1s = nc.dram_tensor("v1s", [IDM, HDM], f16, kind="ExternalInput")
    v2h = nc.dram_tensor("v2h", [HDM, IDM], f16, kind="ExternalInput")
    maskd = nc.dram_tensor("maskd", [3, P, P], f16, kind="ExternalInput")
    out = nc.dram_tensor("out", [NT, P, IDM], f32, kind="ExternalOutput")

    with tile.TileContext(nc) as tc, ExitStack() as ctx:
        singles = ctx.enter_context(tc.tile_pool(name="singles", bufs=1))

        # ---- small inputs first so projections can start with iT slice 0
        wh_sb = singles.tile([P, 4, P], f16)
        nc.sync.dma_start(out=wh_sb, in_=wh.rearrange("(c p) h -> p c h", p=P))
        wl_sb = singles.tile([P, 4, P], f16)
        nc.sync.dma_start(out=wl_sb, in_=wl.rearrange("(c p) h -> p c h", p=P))
        v1_sb = singles.tile([P, 4, HDM], f16)
        nc.sync.dma_start(out=v1_sb, in_=v1s.rearrange("(c p) h -> p c h", p=P))
        v2_sb = singles.tile([HDM, IDM], f16)
        nc.sync.dma_start(out=v2_sb, in_=v2h[:, :])
        maskd_sb = singles.tile([P, 3, P], f16)
        nc.sync.dma_start(out=maskd_sb, in_=maskd.rearrange("m p q -> p m q"))
        ih_sb = singles.tile([P, 4, LN], f16)
        ih_r = ih.rearrange("(c p) n -> p c n", p=P)
        il_sb = singles.tile([P, 4, LN], f16)
        il_r = il.rearrange("(c p) n -> p c n", p=P)
        for j in range(LN // SC):
            nc.sync.dma_start(
                out=ih_sb[:, :, j * SC:(j + 1) * SC],
                in_=ih_r[:, :, j * SC:(j + 1) * SC])
            nc.sync.dma_start(
                out=il_sb[:, :, j * SC:(j + 1) * SC],
                in_=il_r[:, :, j * SC:(j + 1) * SC])

        id16 = singles.tile([P, P], f16)
        from concourse.masks import make_identity
        make_identity(nc, id16)

        # f16x2 score operands. KA row 64 = ones, QA row 64 = -(m+32).
        KA = singles.tile([HDM + 1, LN], f16)      # [Khi; 1]
        KB = singles.tile([P, LN], f16)            # [Klo; Khi]
        QA = singles.tile([HDM + 1, NT * P], f16)  # [Qhi; -(m+32)]
        QB = singles.tile([P, NT * P], f16)        # [Qhi; Qlo]
        nc.vector.memset(KA[HDM:HDM + 1, :], 1.0)
        pv_sb = singles.tile([P, NKT, HDM + 1], f16)
        nc.vector.memset(pv_sb[:, :, HDM:HDM + 1], 1.0)

        sb = ctx.enter_context(tc.tile_pool(name="sbwork", bufs=3))
        stat = ctx.enter_context(tc.tile_pool(name="stat", bufs=2))
        # PSUM bank map (8 banks): a/qk(2) pv/pvblk(1) b(2) o(2) tiny(1)
        ppool = ctx.enter_context(tc.tile_pool(name="ppool", bufs=2, space="PSUM"))
        bpool = ctx.enter_context(tc.tile_pool(name="bpool", bufs=2, space="PSUM"))
        opool = ctx.enter_context(tc.tile_pool(name="opool", bufs=2, space="PSUM"))
        tiny = ctx.enter_context(tc.tile_pool(name="tiny", bufs=1, space="PSUM"))
        epool = ctx.enter_context(tc.tile_pool(name="epool", bufs=56))

        def pass_a(t):
            """Row max of tile t (bf16-hi scores) -> -m into QA row 64.
            Causal masking via PE additive mask blocks (maskd[2]=row-major
            tri, maskd[1]=partner); plain DVE reduce_max into mxp columns."""
            fc = t // 2
            dw = 256 if t % 2 == 0 else 512
            off = dw - 256
            lhsT = QA[0:HDM, t * P:(t + 1) * P]
            mxp = stat.tile([P, 10], f32, tag="mxp", name="mxp")
            for c in range(fc):
                aps = ppool.tile([P, SC], f32, tag="a", name="aps")
                nc.tensor.matmul(
                    aps, lhsT=lhsT, rhs=KA[0:HDM, c * SC:(c + 1) * SC],
                    start=True, stop=True)
                nc.vector.reduce_max(
                    out=mxp[:, c:c + 1], in_=aps, axis=mybir.AxisListType.X)
            dps = ppool.tile([P, SC], f32, tag="a", name="dps")
            nc.tensor.matmul(
                dps[:, 0:dw], lhsT=lhsT,
                rhs=KA[0:HDM, fc * SC:fc * SC + dw], start=True, stop=False)
            nc.tensor.matmul(
                dps[:, off:off + P], lhsT=id16, rhs=maskd_sb[:, 2, :],
                start=False, stop=False)
            nc.tensor.matmul(
                dps[:, off + P:off + 2 * P], lhsT=id16, rhs=maskd_sb[:, 1, :],
                start=False, stop=True)
            nc.vector.reduce_max(
                out=mxp[:, fc:fc + 1], in_=dps[:, 0:dw],
                axis=mybir.AxisListType.X)
            m = stat.tile([P, 1], f32, tag="m", name="m")
            nc.vector.reduce_max(
                out=m, in_=mxp[:, 0:fc + 1], axis=mybir.AxisListType.X)
            mneg = stat.tile([P, 1], f16, tag="mneg", name="mneg")
            nc.scalar.activation(
                out=mneg, in_=m, func=mybir.ActivationFunctionType.Copy,
                scale=-1.0, bias=-32.0)
            mt = tiny.tile([1, P], f16, tag="tp", name="mt")
            nc.tensor.transpose(mt, mneg, id16)
            nc.scalar.copy(out=QA[HDM:HDM + 1, t * P:(t + 1) * P], in_=mt)

        def b_work(gp, kt_lo, kt_hi):
            """bf16x2 transposed scores + exp for group gp (tiles
            4gp..4gp+3), then per-tile O accumulation from saved E tiles
            (each tile's O owns its psum bank for the whole group)."""
            tstart = 4 * gp
            q0 = tstart * P
            gw = 4 * P
            es = []
            for kt in range(kt_lo, kt_hi):
                t0l = max(0, kt // 2 - tstart)  # first covered group tile
                co = t0l * P
                kc = slice(kt * P, (kt + 1) * P)
                qc = slice(q0 + co, q0 + gw)
                bps = bpool.tile([P, SC], f32, tag="b", name="bps")
                in_band = kt >= 2 * tstart
                nc.tensor.matmul(
                    bps[:, co:gw], lhsT=KA[:, kc], rhs=QA[:, qc],
                    start=True, stop=False)
                nc.tensor.matmul(
                    bps[:, co:gw], lhsT=KB[:, kc], rhs=QB[:, qc],
                    start=False, stop=not in_band)
                if in_band:                    # diag tri / partner mask add
                    tl = kt // 2 - tstart
                    mi = kt % 2                # 0: tri mask, 1: partner mask
                    nc.tensor.matmul(
                        bps[:, tl * P:(tl + 1) * P], lhsT=id16,
                        rhs=maskd_sb[:, mi, :], start=False, stop=True)
                e_sb = epool.tile([P, SC], f16, tag="e", name="e_sb")
                nc.scalar.activation(
                    out=e_sb[:, co:gw], in_=bps[:, co:gw],
                    func=mybir.ActivationFunctionType.Exp, scale=0.125)
                es.append((kt, t0l, e_sb))
            for tl in range(4):
                t_abs = tstart + tl
                o_ps = opool.tile([P, HDM + 1], f32, tag="o", name="o_ps")
                last = 2 * t_abs + 1
                for kt, t0l, e_sb in es:
                    if tl < t0l or kt > last:
                        continue
                    nc.tensor.matmul(
                        o_ps, lhsT=e_sb[:, tl * P:(tl + 1) * P],
                        rhs=pv_sb[:, kt, :],
                        start=(kt == 0), stop=(kt == last))
                # finalize tile t_abs
                linv = stat.tile([P, 1], f32, tag="linv", name="linv")
                nc.vector.reciprocal(linv, o_ps[:, HDM:HDM + 1])
                on_sb = sb.tile([P, HDM], f16, tag="on", name="on_sb")
                nc.vector.tensor_scalar_mul(on_sb, o_ps[:, 0:HDM], linv)
                ot_ps = tiny.tile([HDM, P], f16, tag="tp", name="ot_ps")
                nc.tensor.transpose(ot_ps, on_sb, id16)
                ot_sb = sb.tile([HDM, P], f16, tag="otsb", name="ot_sb")
                nc.vector.tensor_copy(ot_sb, ot_ps)
                f_ps = ppool.tile([P, IDM], f32, tag="pv", bufs=1, name="f_ps")
                nc.tensor.matmul(
                    f_ps, lhsT=ot_sb, rhs=v2_sb, start=True, stop=True)
                f_sb = sb.tile([P, IDM], f32, tag="fsb", name="f_sb")
                nc.vector.tensor_copy(f_sb, f_ps)
                nc.sync.dma_start(out=out[t_abs], in_=f_sb)

        # ---- main loop: fp32 proj slice c -> bf16x2 extraction -> pass A
        for c in range(LN // SC):
            sl = slice(c * SC, (c + 1) * SC)
            ps = ppool.tile([P, SC], f32, tag="a", name="ps")
            for d in range(4):   # 3-term compensated f16 projection
                nc.tensor.matmul(
                    ps, lhsT=wh_sb[:, d, :], rhs=ih_sb[:, d, sl],
                    start=(d == 0), stop=False)
                nc.tensor.matmul(
                    ps, lhsT=wh_sb[:, d, :], rhs=il_sb[:, d, sl],
                    start=False, stop=False)
                nc.tensor.matmul(
                    ps, lhsT=wl_sb[:, d, :], rhs=ih_sb[:, d, sl],
                    start=False, stop=(d == 3))
            # K extraction: hi (ACT) -> lo = ps - hi (DVE) -> hi replica (DVE)
            nc.scalar.copy(out=KA[0:HDM, sl], in_=ps[HDM:P, :])
            nc.vector.tensor_tensor(
                out=KB[0:HDM, sl], in0=ps[HDM:P, :], in1=KA[0:HDM, sl],
                op=mybir.AluOpType.subtract)
            nc.vector.tensor_copy(KB[HDM:P, sl], KA[0:HDM, sl])
            # Q extraction for own tiles 2c, 2c+1 (even psum block positions)
            for u in range(2):
                t = 2 * c + u
                qsl = slice(t * P, (t + 1) * P)
                psl = slice(2 * u * P, (2 * u + 1) * P)
                nc.scalar.copy(out=QA[0:HDM, qsl], in_=ps[0:HDM, psl])
                nc.vector.tensor_tensor(
                    out=QB[HDM:P, qsl], in0=ps[0:HDM, psl],
                    in1=QA[0:HDM, qsl], op=mybir.AluOpType.subtract)
                nc.vector.tensor_copy(QB[0:HDM, qsl], QA[0:HDM, qsl])
            # pv projection direct [key, hdm] layout from bf16 i
            for u in range(4):
                kt = 4 * c + u
                psv = ppool.tile([P, HDM], f32, tag="pv", bufs=1, name="psv")
                for d in range(4):
                    nc.tensor.matmul(
                        psv, lhsT=ih_sb[:, d, kt * P:(kt + 1) * P],
                        rhs=v1_sb[:, d, :], start=(d == 0), stop=(d == 3))
                nc.scalar.copy(out=pv_sb[:, kt, 0:HDM], in_=psv)
            pass_a(2 * c)
            pass_a(2 * c + 1)
            # B group gp woven in as soon as its row maxes complete
            if c % 2 == 1:
                gp = (c - 1) // 2
                b_work(gp, 0, 8 * gp + 8)

    nc.finalize()
    return nc


def make_core_inputs_v3(inputs):
    i = np.asarray(inputs["i"], dtype=np.float32)
    q = np.asarray(inputs["q"], dtype=np.float32)
    k = np.asarray(inputs["k"], dtype=np.float32)
    v1 = np.asarray(inputs["v1"], dtype=np.float32)
    v2 = np.asarray(inputs["v2"], dtype=np.float32)
    v2h = np.ascontiguousarray(v2.astype(np.float16))
    v1b = np.ascontiguousarray(v1.astype(np.float16))
    wqk = np.concatenate([q, k], axis=1)
    wh = wqk.astype(np.float16)
    wl = (wqk - wh.astype(np.float32)).astype(np.float16)
    iota = np.arange(P, dtype=np.float32)
    # tri mask for S^T diag block: valid iff k_local <= q_local
    tri = np.where(iota[:, None] <= iota[None, :], 0.0, MASK_NEG).astype(np.float16)
    # row-major tri for pass A: valid iff k_local <= q_local (q on partitions)
    tri_r = np.where(iota[None, :] <= iota[:, None], 0.0, MASK_NEG).astype(np.float16)
    in_maps = []
    for core in range(8):
        b, h = core // 2, core % 2
        perm_blocks = []
        for j in range(NT):
            perm_blocks += [2 * j + h, 2 * j + 1 - h]
        cols = np.concatenate(
            [np.arange(P * g, P * g + P) for g in perm_blocks])
        iTp = np.ascontiguousarray(i[b].T[:, cols])      # [512, 4096]
        maskd = np.stack([
            tri,
            np.full((P, P), 0.0 if h == 1 else MASK_NEG, dtype=np.float16),
            tri_r,
        ]).astype(np.float16)
        ihp = iTp.astype(np.float16)
        ilp = (iTp - ihp.astype(np.float32)).astype(np.float16)
        in_maps.append({
            "ih": np.ascontiguousarray(ihp), "il": np.ascontiguousarray(ilp),
            "wh": np.ascontiguousarray(wh), "wl": np.ascontiguousarray(wl),
            "v1s": v1b, "v2h": v2h,
            "maskd": np.ascontiguousarray(maskd),
        })
    return in_maps


_NC_CACHE = {}


def run_v3(inputs, **spmd_kwargs):
    if "v3" not in _NC_CACHE:
        _NC_CACHE["v3"] = build_nc_v3()
    nc = _NC_CACHE["v3"]
    in_maps = make_core_inputs_v3(inputs)
    res = run_bass_kernel_spmd(nc, in_maps, core_ids=list(range(8)), **spmd_kwargs)
    full = np.zeros((B, LN, IDM), dtype=np.float32)
    for core in range(8):
        b, h = core // 2, core % 2
        o = res.results[core]["out"]                 # [16, 128, 512] f32
        for t in range(NT):
            g = 2 * t + h
            full[b, g * P:(g + 1) * P] = o[t]
    return full, res


def kernel(**inputs):
    full, _ = run_v3(inputs)
    return full



# revision 15
# speedup vs baseline: 1.1353x; 1.1353x over previous
"""Trainium2 Bass kernel: single-head causal attention with low-rank V.

Math (per batch b):
  Q = i@q, K = i@k                  [4096, 64]
  S = Q K^T  (causal mask, /8)      [4096, 4096]
  A = softmax(S)
  out = A @ ((i@v1) @ v2) = ((A @ (i@v1)) / l) @ v2   (low-rank reassociation)

Sharding: 8 cores = 4 batches x 2 halves. Core half h owns query tiles
g = 2t+h (t=0..15). One SPMD program; per-core differences are data only
(column-permuted i^T so own tiles sit at even block positions, plus two
[128,128] mask blocks and a row-major tri block).

v4 design (pair-groups + engine rebalance):
  P1: packed QK^T projection (wq|wk) 3-term compensated f16; pv = (i@v1)
      direct f16, 4 kt batched per psum tile.
  P2 (pass A, row-major): S chunks -> row max m via DVE reduce_max;
      final combine + mneg on Pool; -m written into row 64 of Q^T.
  P3 (pass B, transposed, PAIR groups of 2 tiles = 256 q cols): per kt
      two matmuls m1 = [Khi;1]^T[Qhi;-m], m2 = [Klo;Khi]^T[Qhi;Qlo];
      TWO kt blocks packed side-by-side in one psum bank -> ONE
      [128,512] exp per kt pair. Groups woven with a 1-chunk lag so
      group g's scores interleave with chunk g+1's pass-A matmuls
      (no PE head-of-line blocking on DVE reduces).
  O[q,65] += E^T^T @ [pv|1]; finalize: O/l -> f16 -> transpose -> @ v2
      -> DMA straight from psum.
"""

import sys

if "/opt/trn_rl_repo" not in sys.path:
    sys.path.insert(0, "/opt/trn_rl_repo")

from contextlib import ExitStack

import numpy as np

import concourse.bass as bass
import concourse.tile as tile
from concourse import bacc
from concourse import mybir
from concourse.bass_utils import run_bass_kernel_spmd

B, LN, IDM, HDM = 4, 4096, 512, 64
P = 128          # partitions / tile rows
NT = 16          # query tiles per core
NKT = 32         # key (pos-)tiles per batch
SC = 512         # key chunk (1 psum bank of fp32)
MASK_NEG = -60000.0  # "-inf" in f16-representable units (pre-scale)


def build_nc_v4():
    f32 = mybir.dt.float32
    f16 = mybir.dt.float16
    nc = bacc.Bacc()

    ih = nc.dram_tensor("ih", [IDM, LN], f16, kind="ExternalInput")
    il = nc.dram_tensor("il", [IDM, LN], f16, kind="ExternalInput")
    wh = nc.dram_tensor("wh", [IDM, P], f16, kind="ExternalInput")
    wl = nc.dram_tensor("wl", [IDM, P], f16, kind="ExternalInput")
    v1s = nc.dram_tensor("v1s", [IDM, HDM], f16, kind="ExternalInput")
    v2h = nc.dram_tensor("v2h", [HDM, IDM], f16, kind="ExternalInput")
    maskd = nc.dram_tensor("maskd", [3, P, P], f16, kind="ExternalInput")
    out = nc.dram_tensor("out", [NT, P, IDM], f32, kind="ExternalOutput")

    with tile.TileContext(nc) as tc, ExitStack() as ctx:
        singles = ctx.enter_context(tc.tile_pool(name="singles", bufs=1))

        # ---- small inputs first so projections can start with iT slice 0
        wh_sb = singles.tile([P, 4, P], f16)
        nc.sync.dma_start(out=wh_sb, in_=wh.rearrange("(c p) h -> p c h", p=P))
        wl_sb = singles.tile([P, 4, P], f16)
        nc.sync.dma_start(out=wl_sb, in_=wl.rearrange("(c p) h -> p c h", p=P))
        v1_sb = singles.tile([P, 4, HDM], f16)
        nc.sync.dma_start(out=v1_sb, in_=v1s.rearrange("(c p) h -> p c h", p=P))
        v2_sb = singles.tile([HDM, IDM], f16)
        nc.sync.dma_start(out=v2_sb, in_=v2h[:, :])
        maskd_sb = singles.tile([P, 3, P], f16)
        nc.sync.dma_start(out=maskd_sb, in_=maskd.rearrange("m p q -> p m q"))
        ih_sb = singles.tile([P, 4, LN], f16)
        ih_r = ih.rearrange("(c p) n -> p c n", p=P)
        il_sb = singles.tile([P, 4, LN], f16)
        il_r = il.rearrange("(c p) n -> p c n", p=P)
        for j in range(LN // SC):
            nc.sync.dma_start(
                out=ih_sb[:, :, j * SC:(j + 1) * SC],
                in_=ih_r[:, :, j * SC:(j + 1) * SC])
            nc.sync.dma_start(
                out=il_sb[:, :, j * SC:(j + 1) * SC],
                in_=il_r[:, :, j * SC:(j + 1) * SC])

        id16 = singles.tile([P, P], f16)
        from concourse.masks import make_identity
        make_identity(nc, id16)

        # f16x2 score operands, one tile per 512-token chunk so the tile
        # dependency tracker never sees false cross-chunk hazards.
        # KA row 64 = ones, QA row 64 = -(m+32).
        NC = LN // SC
        KA = [singles.tile([HDM + 1, SC], f16, name=f"KA{c}")
              for c in range(NC)]                  # [Khi; 1]
        KB = [singles.tile([P, SC], f16, name=f"KB{c}")
              for c in range(NC)]                  # [Klo; Khi]
        QA = [singles.tile([HDM + 1, 2 * P], f16, name=f"QA{c}")
              for c in range(NC)]                  # [Qhi; -(m+32)]
        QB = [singles.tile([P, 2 * P], f16, name=f"QB{c}")
              for c in range(NC)]                  # [Qhi; Qlo]
        for c in range(NC):
            nc.gpsimd.memset(KA[c][HDM:HDM + 1, :], 1.0)
        pv_sb = [singles.tile([P, 4, HDM + 1], f16, name=f"pv{c}")
                 for c in range(NC)]
        for c in range(NC):
            nc.gpsimd.memset(pv_sb[c][:, :, HDM:HDM + 1], 1.0)

        sb = ctx.enter_context(tc.tile_pool(name="sbwork", bufs=3))
        stat = ctx.enter_context(tc.tile_pool(name="stat", bufs=3))
        # PSUM bank map (8 banks): a/proj(2) pv/psv/f(1) b(2) o(2) tiny(1)
        ppool = ctx.enter_context(tc.tile_pool(name="ppool", bufs=2, space="PSUM"))
        bpool = ctx.enter_context(tc.tile_pool(name="bpool", bufs=2, space="PSUM"))
        opool = ctx.enter_context(tc.tile_pool(name="opool", bufs=2, space="PSUM"))
        tiny = ctx.enter_context(tc.tile_pool(name="tiny", bufs=1, space="PSUM"))
        epool = ctx.enter_context(tc.tile_pool(name="epool", bufs=18))

        # ---------- emission helpers (generator-style weaving) ----------

        def proj_chunk(c):
            """Projection + extraction + pv for chunk c. Yields nothing;
            emitted as a unit (cheap, front of chunk)."""
            sl = slice(c * SC, (c + 1) * SC)
            ps = ppool.tile([P, SC], f32, tag="a", name="ps")
            # 3-term compensated f16 projection; ih-only terms first so the
            # il DMA of this slice is off the critical path
            for d in range(4):
                nc.tensor.matmul(
                    ps, lhsT=wh_sb[:, d, :], rhs=ih_sb[:, d, sl],
                    start=(d == 0), stop=False)
                nc.tensor.matmul(
                    ps, lhsT=wl_sb[:, d, :], rhs=ih_sb[:, d, sl],
                    start=False, stop=False)
            for d in range(4):
                nc.tensor.matmul(
                    ps, lhsT=wh_sb[:, d, :], rhs=il_sb[:, d, sl],
                    start=False, stop=(d == 3))
            # K extraction: hi (ACT) -> lo = ps - hi (DVE) -> hi rep (DVE)
            nc.scalar.copy(out=KA[c][0:HDM, :], in_=ps[HDM:P, :])
            nc.vector.tensor_tensor(
                out=KB[c][0:HDM, :], in0=ps[HDM:P, :], in1=KA[c][0:HDM, :],
                op=mybir.AluOpType.subtract)
            nc.vector.tensor_copy(KB[c][HDM:P, :], KA[c][0:HDM, :])
            # Q extraction for own tiles 2c, 2c+1 (even psum block positions)
            for u in range(2):
                qsl = slice(u * P, (u + 1) * P)
                psl = slice(2 * u * P, (2 * u + 1) * P)
                nc.scalar.copy(out=QA[c][0:HDM, qsl], in_=ps[0:HDM, psl])
                nc.vector.tensor_tensor(
                    out=QB[c][HDM:P, qsl], in0=ps[0:HDM, psl],
                    in1=QA[c][0:HDM, qsl], op=mybir.AluOpType.subtract)
                nc.vector.tensor_copy(QB[c][0:HDM, qsl], QA[c][0:HDM, qsl])
            # pv projection: 4 kt batched into one [128, 256] psum tile
            psv = ppool.tile([P, 4 * HDM], f32, tag="pv", bufs=1, name="psv")
            for u in range(4):
                kt = 4 * c + u
                for d in range(4):
                    nc.tensor.matmul(
                        psv[:, u * HDM:(u + 1) * HDM],
                        lhsT=ih_sb[:, d, kt * P:(kt + 1) * P],
                        rhs=v1_sb[:, d, :], start=(d == 0), stop=(d == 3),
                        skip_group_check=True)
            nc.vector.tensor_copy(
                pv_sb[c][:, :, 0:HDM],
                psv.rearrange("p (u h) -> p u h", u=4))

        def pass_a_steps(t):
            """Generator: one yield per PE instruction block of pass A for
            tile t, so the caller can interleave b-score work between psum
            uses (reduce on DVE is ~3x slower than the feeding matmul)."""
            fc = t // 2
            dw = 256 if t % 2 == 0 else 512
            off = dw - 256
            lhsT = QA[fc][0:HDM, (t % 2) * P:(t % 2 + 1) * P]
            mxp = stat.tile([P, 10], f32, tag="mxp", name="mxp")
            for c in range(fc):
                aps = ppool.tile([P, SC], f32, tag="a", name="aps")
                nc.tensor.matmul(
                    aps, lhsT=lhsT, rhs=KA[c][0:HDM, :],
                    start=True, stop=True)
                nc.vector.reduce_max(
                    out=mxp[:, c:c + 1], in_=aps, axis=mybir.AxisListType.X)
                yield
            dps = ppool.tile([P, SC], f32, tag="a", name="dps")
            nc.tensor.matmul(
                dps[:, 0:dw], lhsT=lhsT,
                rhs=KA[fc][0:HDM, 0:dw], start=True, stop=False)
            nc.tensor.matmul(
                dps[:, off:off + P], lhsT=id16, rhs=maskd_sb[:, 2, :],
                start=False, stop=False)
            nc.tensor.matmul(
                dps[:, off + P:off + 2 * P], lhsT=id16, rhs=maskd_sb[:, 1, :],
                start=False, stop=True)
            nc.vector.reduce_max(
                out=mxp[:, fc:fc + 1], in_=dps[:, 0:dw],
                axis=mybir.AxisListType.X)
            yield
            # final combine (sbuf->sbuf) + mneg on Pool; transpose on PE;
            # row write via ACT (psum->sbuf)
            m = stat.tile([P, 1], f32, tag="m", name="m")
            nc.vector.reduce_max(
                out=m, in_=mxp[:, 0:fc + 1], axis=mybir.AxisListType.X)
            mneg = stat.tile([P, 1], f16, tag="mneg", name="mneg")
            nc.gpsimd.tensor_scalar(
                out=mneg, in0=m, scalar1=-1.0, scalar2=-32.0,
                op0=mybir.AluOpType.mult, op1=mybir.AluOpType.add)
            mt = tiny.tile([1, P], f16, tag="tp", name="mt")
            nc.tensor.transpose(mt, mneg, id16)
            nc.scalar.copy(
                out=QA[fc][HDM:HDM + 1, (t % 2) * P:(t % 2 + 1) * P], in_=mt)
            yield

        def b_score_steps(g, kt_hi):
            """Generator for pair-group g (tiles 2g, 2g+1; q cols
            [256g, 256g+256)): per kt PAIR, two score matmuls each + masks
            packed 2-kt-per-bank, one [128,512] exp. One yield per kt."""
            tstart = 2 * g
            es = []
            for ktp in range(0, kt_hi, 2):
                bps = bpool.tile([P, 2 * SC // 2], f32, tag="b", name="bps")
                for hh in range(2):
                    kt = ktp + hh
                    kc = slice((kt % 4) * P, (kt % 4 + 1) * P)
                    o0 = hh * 256
                    in_band = kt >= 2 * tstart
                    nc.tensor.matmul(
                        bps[:, o0:o0 + 256], lhsT=KA[kt // 4][:, kc],
                        rhs=QA[g][:, :],
                        start=True, stop=False, skip_group_check=True)
                    nc.tensor.matmul(
                        bps[:, o0:o0 + 256], lhsT=KB[kt // 4][:, kc],
                        rhs=QB[g][:, :],
                        start=False, stop=not in_band, skip_group_check=True)
                    if in_band:                # diag tri / partner mask add
                        tl = kt // 2 - tstart
                        mi = kt % 2            # 0: tri mask, 1: partner mask
                        nc.tensor.matmul(
                            bps[:, o0 + tl * P:o0 + (tl + 1) * P], lhsT=id16,
                            rhs=maskd_sb[:, mi, :], start=False, stop=True,
                            skip_group_check=True)
                    yield
                e_sb = epool.tile([P, 2 * 256], f16, tag="e", name="e_sb")
                nc.scalar.activation(
                    out=e_sb, in_=bps,
                    func=mybir.ActivationFunctionType.Exp, scale=0.125)
                es.append(e_sb)
            # O accumulation + finalize per tile
            for tl in range(2):
                t_abs = tstart + tl
                o_ps = opool.tile([P, HDM + 1], f32, tag="o", name="o_ps")
                last = 2 * t_abs + 1
                for kt in range(0, last + 1):
                    e_sb = es[kt // 2]
                    hh = kt % 2
                    nc.tensor.matmul(
                        o_ps, lhsT=e_sb[:, hh * 256 + tl * P:
                                        hh * 256 + (tl + 1) * P],
                        rhs=pv_sb[kt // 4][:, kt % 4, :],
                        start=(kt == 0), stop=(kt == last))
                # finalize tile t_abs
                linv = stat.tile([P, 1], f32, tag="linv", name="linv")
                nc.vector.reciprocal(linv, o_ps[:, HDM:HDM + 1])
                on_sb = sb.tile([P, HDM], f16, tag="on", name="on_sb")
                nc.scalar.activation(
                    out=on_sb, in_=o_ps[:, 0:HDM],
                    func=mybir.ActivationFunctionType.Copy, scale=linv)
                ot_ps = tiny.tile([HDM, P], f16, tag="tp", name="ot_ps")
                nc.tensor.transpose(ot_ps, on_sb, id16)
                ot_sb = sb.tile([HDM, P], f16, tag="otsb", name="ot_sb")
                nc.vector.tensor_copy(ot_sb, ot_ps)
                f_ps = ppool.tile([P, IDM], f32, tag="pv", bufs=1, name="f_ps")
                nc.tensor.matmul(
                    f_ps, lhsT=ot_sb, rhs=v2_sb, start=True, stop=True)
                f_sb = sb.tile([P, IDM], f32, tag="fsb", name="f_sb")
                if tl == 0:
                    nc.vector.tensor_copy(f_sb, f_ps)
                else:
                    nc.scalar.copy(out=f_sb, in_=f_ps)
                nc.sync.dma_start(out=out[t_abs], in_=f_sb)
                yield

        def drain(gen):
            if gen is not None:
                for _ in gen:
                    pass

        # ---- main loop: chunk c does proj/extraction/pv + pass A of tiles
        # 2c, 2c+1 interleaved with pair-group (c-1) score/exp/O work.
        # The b work is paced evenly across the pass-A psum slots so PE
        # always has score matmuls to run while DVE drains reduces.
        bgen = None
        for c in range(LN // SC):
            proj_chunk(c)
            nslots = 2 * (c + 2)              # pass-A yields this chunk
            nitems = (4 * (c - 1) + 6) if c > 0 else 0
            rate = nitems / nslots if nslots else 0.0
            frac = 0.0
            for t in (2 * c, 2 * c + 1):
                for _ in pass_a_steps(t):
                    frac += rate
                    while frac >= 1.0 and bgen is not None:
                        next(bgen, None)
                        frac -= 1.0
            drain(bgen)
            bgen = b_score_steps(c, 4 * c + 4)
        drain(bgen)

    nc.finalize()
    return nc


def make_core_inputs_v4(inputs):
    i = np.asarray(inputs["i"], dtype=np.float32)
    q = np.asarray(inputs["q"], dtype=np.float32)
    k = np.asarray(inputs["k"], dtype=np.float32)
    v1 = np.asarray(inputs["v1"], dtype=np.float32)
    v2 = np.asarray(inputs["v2"], dtype=np.float32)
    v2h = np.ascontiguousarray(v2.astype(np.float16))
    v1b = np.ascontiguousarray(v1.astype(np.float16))
    wqk = np.concatenate([q, k], axis=1)
    wh = wqk.astype(np.float16)
    wl = (wqk - wh.astype(np.float32)).astype(np.float16)
    iota = np.arange(P, dtype=np.float32)
    # tri mask for S^T diag block: valid iff k_local <= q_local
    tri = np.where(iota[:, None] <= iota[None, :], 0.0, MASK_NEG).astype(np.float16)
    # row-major tri for pass A: valid iff k_local <= q_local (q on partitions)
    tri_r = np.where(iota[None, :] <= iota[:, None], 0.0, MASK_NEG).astype(np.float16)
    in_maps = []
    for core in range(8):
        b, h = core // 2, core % 2
        perm_blocks = []
        for j in range(NT):
            perm_blocks += [2 * j + h, 2 * j + 1 - h]
        cols = np.concatenate(
            [np.arange(P * g, P * g + P) for g in perm_blocks])
        iTp = np.ascontiguousarray(i[b].T[:, cols])      # [512, 4096]
        maskd = np.stack([
            tri,
            np.full((P, P), 0.0 if h == 1 else MASK_NEG, dtype=np.float16),
            tri_r,
        ]).astype(np.float16)
        ihp = iTp.astype(np.float16)
        ilp = (iTp - ihp.astype(np.float32)).astype(np.float16)
        in_maps.append({
            "ih": np.ascontiguousarray(ihp), "il": np.ascontiguousarray(ilp),
            "wh": np.ascontiguousarray(wh), "wl": np.ascontiguousarray(wl),
            "v1s": v1b, "v2h": v2h,
            "maskd": np.ascontiguousarray(maskd),
        })
    return in_maps


_NC_CACHE = {}


def run_v3(inputs, **spmd_kwargs):
    if "v4" not in _NC_CACHE:
        _NC_CACHE["v4"] = build_nc_v4()
    nc = _NC_CACHE["v4"]
    in_maps = make_core_inputs_v4(inputs)
    res = run_bass_kernel_spmd(nc, in_maps, core_ids=list(range(8)), **spmd_kwargs)
    full = np.zeros((B, LN, IDM), dtype=np.float32)
    for core in range(8):
        b, h = core // 2, core % 2
        o = res.results[core]["out"]                 # [16, 128, 512] f32
        for t in range(NT):
            g = 2 * t + h
            full[b, g * P:(g + 1) * P] = o[t]
    return full, res


def kernel(**inputs):
    full, _ = run_v3(inputs)
    return full


# revision 31
# speedup vs baseline: 1.1936x; 1.0514x over previous
"""Trainium2 Bass kernel: single-head causal attention with low-rank V.

Math (per batch b):
  Q = i@q, K = i@k                  [4096, 64]
  S = Q K^T  (causal mask, /8)      [4096, 4096]
  A = softmax(S)
  out = A @ ((i@v1) @ v2) = ((A @ (i@v1)) / l) @ v2   (low-rank reassociation)

Sharding: 8 cores = 4 batches x 2 halves. Core half h owns query tiles
g = 2t+h (t=0..15). One SPMD program; per-core differences are data only
(column-permuted i^T so own tiles sit at even block positions, plus two
[128,128] mask blocks and a row-major tri block).

v4 design (pair-groups + engine rebalance):
  P1: packed QK^T projection (wq|wk) 3-term compensated f16; pv = (i@v1)
      direct f16, 4 kt batched per psum tile.
  P2 (pass A, row-major): S chunks -> row max m via DVE reduce_max;
      final combine + mneg on Pool; -m written into row 64 of Q^T.
  P3 (pass B, transposed, PAIR groups of 2 tiles = 256 q cols): per kt
      two matmuls m1 = [Khi;1]^T[Qhi;-m], m2 = [Klo;Khi]^T[Qhi;Qlo];
      TWO kt blocks packed side-by-side in one psum bank -> ONE
      [128,512] exp per kt pair. Groups woven with a 1-chunk lag so
      group g's scores interleave with chunk g+1's pass-A matmuls
      (no PE head-of-line blocking on DVE reduces).
  O[q,65] += E^T^T @ [pv|1]; finalize: O/l -> f16 -> transpose -> @ v2
      -> DMA straight from psum.
"""

import sys

if "/opt/trn_rl_repo" not in sys.path:
    sys.path.insert(0, "/opt/trn_rl_repo")

from contextlib import ExitStack

import numpy as np

import concourse.bass as bass
import concourse.tile as tile
from concourse import bacc
from concourse import mybir
from concourse.bass_utils import run_bass_kernel_spmd

B, LN, IDM, HDM = 4, 4096, 512, 64
P = 128          # partitions / tile rows
NT = 16          # query tiles per core
NKT = 32         # key (pos-)tiles per batch
SC = 512         # key chunk (1 psum bank of fp32)
MASK_NEG = -60000.0  # "-inf" in f16-representable units (pre-scale)


def build_nc_v4():
    f32 = mybir.dt.float32
    f16 = mybir.dt.float16
    nc = bacc.Bacc()

    ih = nc.dram_tensor("ih", [IDM, LN], f16, kind="ExternalInput")
    il = nc.dram_tensor("il", [IDM, LN], f16, kind="ExternalInput")
    wh = nc.dram_tensor("wh", [IDM, P], f16, kind="ExternalInput")
    wl = nc.dram_tensor("wl", [IDM, P], f16, kind="ExternalInput")
    v1s = nc.dram_tensor("v1s", [IDM, HDM], f16, kind="ExternalInput")
    v2h = nc.dram_tensor("v2h", [HDM, IDM], f16, kind="ExternalInput")
    maskd = nc.dram_tensor("maskd", [3, P, P], f16, kind="ExternalInput")
    out = nc.dram_tensor("out", [NT, P, IDM], f32, kind="ExternalOutput")

    with tile.TileContext(nc) as tc, ExitStack() as ctx:
        singles = ctx.enter_context(tc.tile_pool(name="singles", bufs=1))

        # ---- small inputs first so projections can start with iT slice 0
        wh_sb = singles.tile([P, 4, P], f16)
        nc.sync.dma_start(out=wh_sb, in_=wh.rearrange("(c p) h -> p c h", p=P))
        wl_sb = singles.tile([P, 4, P], f16)
        nc.sync.dma_start(out=wl_sb, in_=wl.rearrange("(c p) h -> p c h", p=P))
        ih_sb = singles.tile([P, 4, LN], f16)
        ih_r = ih.rearrange("(c p) n -> p c n", p=P)
        il_sb = singles.tile([P, 4, LN], f16)
        il_r = il.rearrange("(c p) n -> p c n", p=P)
        # slice 0 right after the projection weights so proj(0) starts ASAP
        nc.sync.dma_start(out=ih_sb[:, :, 0:SC], in_=ih_r[:, :, 0:SC])
        nc.sync.dma_start(out=il_sb[:, :, 0:SC], in_=il_r[:, :, 0:SC])
        v1_sb = singles.tile([P, 4, HDM], f16)
        nc.sync.dma_start(out=v1_sb, in_=v1s.rearrange("(c p) h -> p c h", p=P))
        v2_sb = singles.tile([HDM, IDM], f16)
        nc.sync.dma_start(out=v2_sb, in_=v2h[:, :])
        maskd_sb = singles.tile([P, 3, P], f16)
        nc.sync.dma_start(out=maskd_sb, in_=maskd.rearrange("m p q -> p m q"))
        for j in range(1, LN // SC):
            nc.sync.dma_start(
                out=ih_sb[:, :, j * SC:(j + 1) * SC],
                in_=ih_r[:, :, j * SC:(j + 1) * SC])
            nc.sync.dma_start(
                out=il_sb[:, :, j * SC:(j + 1) * SC],
                in_=il_r[:, :, j * SC:(j + 1) * SC])

        id16 = singles.tile([P, P], f16)
        from concourse.masks import make_identity
        make_identity(nc, id16)

        # f16x2 score operands, one tile per 512-token chunk so the tile
        # dependency tracker never sees false cross-chunk hazards.
        # KA row 64 = ones, QA row 64 = -(m+32).
        NC = LN // SC
        KA = [singles.tile([HDM + 1, SC], f16, name=f"KA{c}")
              for c in range(NC)]                  # [Khi; 1]
        KB = [singles.tile([P, SC], f16, name=f"KB{c}")
              for c in range(NC)]                  # [Klo; Khi]
        QA = [singles.tile([HDM + 1, 2 * P], f16, name=f"QA{c}")
              for c in range(NC)]                  # [Qhi; -(m+32)]
        QB = [singles.tile([P, 2 * P], f16, name=f"QB{c}")
              for c in range(NC)]                  # [Qhi; Qlo]
        for c in range(NC):
            nc.gpsimd.memset(KA[c][HDM:HDM + 1, :], 1.0)
        pv_sb = [singles.tile([P, 4, HDM + 1], f16, name=f"pv{c}")
                 for c in range(NC)]
        for c in range(NC):
            nc.gpsimd.memset(pv_sb[c][:, :, HDM:HDM + 1], 1.0)

        sb = ctx.enter_context(tc.tile_pool(name="sbwork", bufs=3))
        stat = ctx.enter_context(tc.tile_pool(name="stat", bufs=3))
        # PSUM bank map (8 banks): a/proj(2) pv/psv/f(1) b(2x2) o(1) tiny(1)
        # b tiles are [128,1024] spanning two banks: FOUR kt score blocks
        # per tile -> one [128,1024] exp per 4 kt. o holds BOTH group
        # tiles' accumulators in one bank (concurrent psum groups per
        # bank are per-address on HW).
        ppool = ctx.enter_context(tc.tile_pool(name="ppool", bufs=2, space="PSUM"))
        bpool = ctx.enter_context(tc.tile_pool(name="bpool", bufs=2, space="PSUM"))
        opool = ctx.enter_context(tc.tile_pool(name="opool", bufs=1, space="PSUM"))
        tiny = ctx.enter_context(tc.tile_pool(name="tiny", bufs=1, space="PSUM"))
        epool = ctx.enter_context(tc.tile_pool(name="epool", bufs=10))

        # shared o/psv bank: o accumulators for both group tiles at
        # [:, 0, 0:65] / [:, 1... wait o uses om[:, 0:65] and om[:, 128:193];
        # psv occupies the second half as a [128, 4, 64] view.
        om = opool.tile([P, 4 * P], f32, tag="om", name="om")
        om_r = om.rearrange("p (x u h) -> p x u h", x=2, u=4)

        # ---------- emission helpers (generator-style weaving) ----------

        def proj_chunk(c):
            """Generator: projection + extraction + pv for chunk c, with
            yields between matmul groups so leftover b work from the
            previous chunk can interleave instead of stalling on the exp
            cadence."""
            sl = slice(c * SC, (c + 1) * SC)
            ps = ppool.tile([P, SC], f32, tag="a", name="ps")
            # 3-term compensated f16 projection; ih-only terms first so the
            # il DMA of this slice is off the critical path
            for d in range(4):
                nc.tensor.matmul(
                    ps, lhsT=wh_sb[:, d, :], rhs=ih_sb[:, d, sl],
                    start=(d == 0), stop=False)
                nc.tensor.matmul(
                    ps, lhsT=wl_sb[:, d, :], rhs=ih_sb[:, d, sl],
                    start=False, stop=False)
                yield
            for d in range(4):
                nc.tensor.matmul(
                    ps, lhsT=wh_sb[:, d, :], rhs=il_sb[:, d, sl],
                    start=False, stop=(d == 3))
                yield
            # K extraction: hi (ACT) -> lo = ps - hi (DVE) -> hi rep (DVE)
            nc.scalar.copy(out=KA[c][0:HDM, :], in_=ps[HDM:P, :])
            nc.vector.tensor_tensor(
                out=KB[c][0:HDM, :], in0=ps[HDM:P, :], in1=KA[c][0:HDM, :],
                op=mybir.AluOpType.subtract)
            nc.vector.tensor_copy(KB[c][HDM:P, :], KA[c][0:HDM, :])
            # Q extraction for own tiles 2c, 2c+1 (even psum block positions)
            for u in range(2):
                qsl = slice(u * P, (u + 1) * P)
                psl = slice(2 * u * P, (2 * u + 1) * P)
                nc.scalar.copy(out=QA[c][0:HDM, qsl], in_=ps[0:HDM, psl])
                nc.vector.tensor_tensor(
                    out=QB[c][HDM:P, qsl], in0=ps[0:HDM, psl],
                    in1=QA[c][0:HDM, qsl], op=mybir.AluOpType.subtract)
                nc.vector.tensor_copy(QB[c][0:HDM, qsl], QA[c][0:HDM, qsl])
            # pv projection: 4 kt batched into the psv region of the shared
            # o/psv psum bank
            for u in range(4):
                kt = 4 * c + u
                for d in range(4):
                    nc.tensor.matmul(
                        om_r[:, 1, u, :],
                        lhsT=ih_sb[:, d, kt * P:(kt + 1) * P],
                        rhs=v1_sb[:, d, :], start=(d == 0), stop=(d == 3),
                        skip_group_check=True)
                yield
            nc.scalar.copy(out=pv_sb[c][:, :, 0:HDM], in_=om_r[:, 1, :, :])

        def pass_a_steps(t):
            """Generator: one yield per PE instruction block of pass A for
            tile t, so the caller can interleave b-score work between psum
            uses (reduce on DVE is ~3x slower than the feeding matmul)."""
            fc = t // 2
            dw = 256 if t % 2 == 0 else 512
            off = dw - 256
            lhsT = QA[fc][0:HDM, (t % 2) * P:(t % 2 + 1) * P]
            mxp = stat.tile([P, 10], f32, tag="mxp", name="mxp")
            for c in range(fc):
                aps = ppool.tile([P, SC], f32, tag="a", name="aps")
                nc.tensor.matmul(
                    aps, lhsT=lhsT, rhs=KA[c][0:HDM, :],
                    start=True, stop=True)
                nc.vector.reduce_max(
                    out=mxp[:, c:c + 1], in_=aps, axis=mybir.AxisListType.X)
                yield
            dps = ppool.tile([P, SC], f32, tag="a", name="dps")
            nc.tensor.matmul(
                dps[:, 0:dw], lhsT=lhsT,
                rhs=KA[fc][0:HDM, 0:dw], start=True, stop=False)
            nc.tensor.matmul(
                dps[:, off:off + P], lhsT=id16, rhs=maskd_sb[:, 2, :],
                start=False, stop=False)
            nc.tensor.matmul(
                dps[:, off + P:off + 2 * P], lhsT=id16, rhs=maskd_sb[:, 1, :],
                start=False, stop=True)
            nc.vector.reduce_max(
                out=mxp[:, fc:fc + 1], in_=dps[:, 0:dw],
                axis=mybir.AxisListType.X)
            yield
            # final combine (sbuf->sbuf) + mneg on Pool; transpose on PE;
            # row write via ACT (psum->sbuf)
            m = stat.tile([P, 1], f32, tag="m", name="m")
            nc.vector.reduce_max(
                out=m, in_=mxp[:, 0:fc + 1], axis=mybir.AxisListType.X)
            mneg = stat.tile([P, 1], f16, tag="mneg", name="mneg")
            nc.gpsimd.tensor_scalar(
                out=mneg, in0=m, scalar1=-1.0, scalar2=-32.0,
                op0=mybir.AluOpType.mult, op1=mybir.AluOpType.add)
            mt = tiny.tile([1, P], f16, tag="tp", name="mt")
            nc.tensor.transpose(mt, mneg, id16)
            # on DVE: lands right behind this tile's final reduce in the
            # FIFO, not behind the previous group's exp queue on ACT
            nc.vector.tensor_copy(
                QA[fc][HDM:HDM + 1, (t % 2) * P:(t % 2 + 1) * P], mt)
            yield

        def b_score_steps(tstart, ntiles, kt_hi):
            """Generator for group of `ntiles` tiles starting at `tstart`
            (q width ntiles*128): per kt two score matmuls + masks, kt
            blocks packed into [128,1024] two-bank psum tiles, one exp per
            psum tile. One yield per kt."""
            qw = ntiles * P
            kpt = 1024 // qw                   # kt blocks per psum tile
            qsl = slice((tstart % 2) * P, (tstart % 2) * P + qw)
            gq = tstart // 2
            es = []
            for ktq in range(0, kt_hi, kpt):
                nk = min(kpt, kt_hi - ktq)
                bps = bpool.tile([P, 1024], f32, tag="b", name="bps")
                for hh in range(nk):
                    kt = ktq + hh
                    kc = slice((kt % 4) * P, (kt % 4 + 1) * P)
                    o0 = hh * qw
                    in_band = kt >= 2 * tstart
                    nc.tensor.matmul(
                        bps[:, o0:o0 + qw], lhsT=KA[kt // 4][:, kc],
                        rhs=QA[gq][:, qsl],
                        start=True, stop=False, skip_group_check=True)
                    nc.tensor.matmul(
                        bps[:, o0:o0 + qw], lhsT=KB[kt // 4][:, kc],
                        rhs=QB[gq][:, qsl],
                        start=False, stop=not in_band, skip_group_check=True)
                    if in_band:                # diag tri / partner mask add
                        tl = kt // 2 - tstart
                        mi = kt % 2            # 0: tri mask, 1: partner mask
                        nc.tensor.matmul(
                            bps[:, o0 + tl * P:o0 + (tl + 1) * P], lhsT=id16,
                            rhs=maskd_sb[:, mi, :], start=False, stop=True,
                            skip_group_check=True)
                    yield
                e_sb = epool.tile([P, 1024], f16, tag="e", name="e_sb")
                nc.scalar.activation(
                    out=e_sb[:, 0:nk * qw], in_=bps[:, 0:nk * qw],
                    func=mybir.ActivationFunctionType.Exp, scale=0.125)
                es.append(e_sb)
            # O accumulation + finalize per tile; both tiles' accumulators
            # live in the shared om bank at 512B-aligned offsets.
            for tl in range(ntiles):
                t_abs = tstart + tl
                o_ps = om[:, tl * P:tl * P + HDM + 1]
                last = 2 * t_abs + 1
                for kt in range(0, last + 1):
                    e_sb = es[kt // kpt]
                    hh = kt % kpt
                    nc.tensor.matmul(
                        o_ps, lhsT=e_sb[:, hh * qw + tl * P:
                                        hh * qw + (tl + 1) * P],
                        rhs=pv_sb[kt // 4][:, kt % 4, :],
                        start=(kt == 0), stop=(kt == last),
                        skip_group_check=True)
                # finalize tile t_abs
                linv = stat.tile([P, 1], f32, tag="linv", name="linv")
                nc.vector.reciprocal(linv, o_ps[:, HDM:HDM + 1])
                on_sb = sb.tile([P, HDM], f16, tag="on", name="on_sb")
                nc.scalar.activation(
                    out=on_sb, in_=o_ps[:, 0:HDM],
                    func=mybir.ActivationFunctionType.Copy, scale=linv)
                ot_ps = tiny.tile([HDM, P], f16, tag="tp", name="ot_ps")
                nc.tensor.transpose(ot_ps, on_sb, id16)
                ot_sb = sb.tile([HDM, P], f16, tag="otsb", name="ot_sb")
                nc.vector.tensor_copy(ot_sb, ot_ps)
                f_ps = bpool.tile([P, IDM], f32, tag="b", name="f_ps")
                nc.tensor.matmul(
                    f_ps, lhsT=ot_sb, rhs=v2_sb, start=True, stop=True)
                f_sb = sb.tile([P, IDM], f32, tag="fsb", name="f_sb")
                if tl == 0:
                    nc.vector.tensor_copy(f_sb, f_ps)
                else:
                    nc.scalar.copy(out=f_sb, in_=f_ps)
                nc.sync.dma_start(out=out[t_abs], in_=f_sb)
                yield

        def drain(gen):
            if gen is not None:
                for _ in gen:
                    pass

        # ---- main loop: chunk c does proj/extraction/pv + pass A of tiles
        # 2c, 2c+1 interleaved with pair-group (c-1) score/exp/O work,
        # paced evenly across the pass-A psum slots.
        from itertools import chain

        def weave(slots, nslots, bgen, nitems):
            rate = nitems / nslots if nslots else 0.0
            frac = 0.0
            for _ in slots:
                frac += rate
                while frac >= 1.0 and bgen is not None:
                    next(bgen, None)
                    frac -= 1.0
            drain(bgen)

        bgen = None
        NCk = LN // SC
        for c in range(NCk - 1):
            slots = chain(proj_chunk(c), pass_a_steps(2 * c),
                          pass_a_steps(2 * c + 1))
            weave(slots, 12 + 2 * (c + 2), bgen,
                  (4 * (c - 1) + 6) if c else 0)
            bgen = b_score_steps(2 * c, 2, 4 * c + 4)
        # last chunk: singleton groups so tile 15's work alone forms the
        # tail and tile 14's work fills the final reduce phase
        c = NCk - 1
        weave(chain(proj_chunk(c), pass_a_steps(2 * c)), 12 + c + 2,
              bgen, 4 * (c - 1) + 6)
        weave(pass_a_steps(2 * c + 1), c + 2,
              b_score_steps(2 * c, 1, 4 * c + 2), 4 * c + 3)
        drain(b_score_steps(2 * c + 1, 1, 4 * c + 4))

    nc.finalize()
    return nc


def make_core_inputs_v4(inputs):
    i = np.asarray(inputs["i"], dtype=np.float32)
    q = np.asarray(inputs["q"], dtype=np.float32)
    k = np.asarray(inputs["k"], dtype=np.float32)
    v1 = np.asarray(inputs["v1"], dtype=np.float32)
    v2 = np.asarray(inputs["v2"], dtype=np.float32)
    v2h = np.ascontiguousarray(v2.astype(np.float16))
    v1b = np.ascontiguousarray(v1.astype(np.float16))
    wqk = np.concatenate([q, k], axis=1)
    wh = wqk.astype(np.float16)
    wl = (wqk - wh.astype(np.float32)).astype(np.float16)
    iota = np.arange(P, dtype=np.float32)
    # tri mask for S^T diag block: valid iff k_local <= q_local
    tri = np.where(iota[:, None] <= iota[None, :], 0.0, MASK_NEG).astype(np.float16)
    # row-major tri for pass A: valid iff k_local <= q_local (q on partitions)
    tri_r = np.where(iota[None, :] <= iota[:, None], 0.0, MASK_NEG).astype(np.float16)
    in_maps = []
    for core in range(8):
        b, h = core // 2, core % 2
        perm_blocks = []
        for j in range(NT):
            perm_blocks += [2 * j + h, 2 * j + 1 - h]
        cols = np.concatenate(
            [np.arange(P * g, P * g + P) for g in perm_blocks])
        iTp = np.ascontiguousarray(i[b].T[:, cols])      # [512, 4096]
        maskd = np.stack([
            tri,
            np.full((P, P), 0.0 if h == 1 else MASK_NEG, dtype=np.float16),
            tri_r,
        ]).astype(np.float16)
        ihp = iTp.astype(np.float16)
        ilp = (iTp - ihp.astype(np.float32)).astype(np.float16)
        in_maps.append({
            "ih": np.ascontiguousarray(ihp), "il": np.ascontiguousarray(ilp),
            "wh": np.ascontiguousarray(wh), "wl": np.ascontiguousarray(wl),
            "v1s": v1b, "v2h": v2h,
            "maskd": np.ascontiguousarray(maskd),
        })
    return in_maps


_NC_CACHE = {}


def run_v3(inputs, **spmd_kwargs):
    if "v4" not in _NC_CACHE:
        _NC_CACHE["v4"] = build_nc_v4()
    nc = _NC_CACHE["v4"]
    in_maps = make_core_inputs_v4(inputs)
    res = run_bass_kernel_spmd(nc, in_maps, core_ids=list(range(8)), **spmd_kwargs)
    full = np.zeros((B, LN, IDM), dtype=np.float32)
    for core in range(8):
        b, h = core // 2, core % 2
        o = res.results[core]["out"]                 # [16, 128, 512] f32
        for t in range(NT):
            g = 2 * t + h
            full[b, g * P:(g + 1) * P] = o[t]
    return full, res


def kernel(**inputs):
    full, _ = run_v3(inputs)
    return full


# revision 63
# speedup vs baseline: 1.2117x; 1.0152x over previous
"""Trainium2 Bass kernel: single-head causal attention with low-rank V.

Math (per batch b):
  Q = i@q, K = i@k                  [4096, 64]
  S = Q K^T  (causal mask, /8)      [4096, 4096]
  A = softmax(S)
  out = A @ ((i@v1) @ v2) = ((A @ (i@v1)) / l) @ v2   (low-rank reassociation)

Sharding: 8 cores = 4 batches x 2 halves. Core half h owns query tiles
g = 2t+h (t=0..15). One SPMD program; per-core differences are data only
(column-permuted i^T so own tiles sit at even block positions, plus two
[128,128] mask blocks and a row-major tri block).

v4 design (pair-groups + engine rebalance):
  P1: packed QK^T projection (wq|wk) 3-term compensated f16; pv = (i@v1)
      direct f16, 4 kt batched per psum tile.
  P2 (pass A, row-major): S chunks -> row max m via DVE reduce_max;
      final combine + mneg on Pool; -m written into row 64 of Q^T.
  P3 (pass B, transposed, PAIR groups of 2 tiles = 256 q cols): per kt
      two matmuls m1 = [Khi;1]^T[Qhi;-m], m2 = [Klo;Khi]^T[Qhi;Qlo];
      TWO kt blocks packed side-by-side in one psum bank -> ONE
      [128,512] exp per kt pair. Groups woven with a 1-chunk lag so
      group g's scores interleave with chunk g+1's pass-A matmuls
      (no PE head-of-line blocking on DVE reduces).
  O[q,65] += E^T^T @ [pv|1]; finalize: O/l -> f16 -> transpose -> @ v2
      -> DMA straight from psum.
"""

import sys

if "/opt/trn_rl_repo" not in sys.path:
    sys.path.insert(0, "/opt/trn_rl_repo")

from contextlib import ExitStack

import numpy as np

import concourse.bass as bass
import concourse.tile as tile
from concourse import bacc
from concourse import mybir
from concourse.bass_utils import run_bass_kernel_spmd

B, LN, IDM, HDM = 4, 4096, 512, 64
P = 128          # partitions / tile rows
NT = 16          # query tiles per core
NKT = 32         # key (pos-)tiles per batch
SC = 512         # key chunk (1 psum bank of fp32)
MASK_NEG = -60000.0  # "-inf" in f16-representable units (pre-scale)


def build_nc_v4():
    f32 = mybir.dt.float32
    f16 = mybir.dt.float16
    nc = bacc.Bacc()

    ihl = nc.dram_tensor("ihl", [2, IDM, LN], f16, kind="ExternalInput")
    whl = nc.dram_tensor("whl", [2, IDM, P], f16, kind="ExternalInput")
    v1s = nc.dram_tensor("v1s", [IDM, HDM], f16, kind="ExternalInput")
    v2h = nc.dram_tensor("v2h", [HDM, IDM], f16, kind="ExternalInput")
    maskd = nc.dram_tensor("maskd", [3, P, P], f16, kind="ExternalInput")
    out = nc.dram_tensor("out", [NT, P, IDM], f32, kind="ExternalOutput")

    with tile.TileContext(nc) as tc, ExitStack() as ctx:
        singles = ctx.enter_context(tc.tile_pool(name="singles", bufs=1))

        # ---- small inputs first so projections can start with iT slice 0
        whl_sb = singles.tile([P, 2, 4, P], f16)
        whl_r = whl.rearrange("t (c p) h -> p t c h", p=P)
        nc.sync.dma_start(out=whl_sb[:, 0:1], in_=whl_r[:, 0:1])
        wh_sb = whl_sb[:, 0]
        wl_sb = whl_sb[:, 1]
        ihl_sb = singles.tile([P, 2, 4, LN], f16)
        ihl_r = ihl.rearrange("t (c p) n -> p t c n", p=P)
        ih_sb = ihl_sb[:, 0]
        il_sb = ihl_sb[:, 1]
        # slice 0 split per hi d-block so proj(0) starts ASAP
        for d in range(4):
            nc.sync.dma_start(
                out=ihl_sb[:, 0:1, d:d + 1, 0:SC],
                in_=ihl_r[:, 0:1, d:d + 1, 0:SC])
        nc.sync.dma_start(out=whl_sb[:, 1:2], in_=whl_r[:, 1:2])
        nc.sync.dma_start(
            out=ihl_sb[:, 1:2, :, 0:SC], in_=ihl_r[:, 1:2, :, 0:SC])
        v1_sb = singles.tile([P, 4, HDM], f16)
        nc.sync.dma_start(out=v1_sb, in_=v1s.rearrange("(c p) h -> p c h", p=P))
        v2_sb = singles.tile([HDM, IDM], f16)
        nc.sync.dma_start(out=v2_sb, in_=v2h[:, :])
        maskd_sb = singles.tile([P, 3, P], f16)
        nc.sync.dma_start(out=maskd_sb, in_=maskd.rearrange("m p q -> p m q"))
        for j in range(1, LN // SC):
            nc.sync.dma_start(
                out=ihl_sb[:, 0:1, :, j * SC:(j + 1) * SC],
                in_=ihl_r[:, 0:1, :, j * SC:(j + 1) * SC])
            nc.sync.dma_start(
                out=ihl_sb[:, 1:2, :, j * SC:(j + 1) * SC],
                in_=ihl_r[:, 1:2, :, j * SC:(j + 1) * SC])

        id16 = singles.tile([P, P], f16)
        from concourse.masks import make_identity
        make_identity(nc, id16)

        # f16x2 score operands, one tile per 512-token chunk so the tile
        # dependency tracker never sees false cross-chunk hazards.
        # KA row 64 = ones, QA row 64 = -(m+32).
        NC = LN // SC
        KA = [singles.tile([HDM + 1, SC], f16, name=f"KA{c}")
              for c in range(NC)]                  # [Khi; 1]
        KB = [singles.tile([P, SC], f16, name=f"KB{c}")
              for c in range(NC)]                  # [Klo; Khi]
        QA = [singles.tile([HDM + 1, 2 * P], f16, name=f"QA{c}")
              for c in range(NC)]                  # [Qhi; -(m+32)]
        QB = [singles.tile([P, 2 * P], f16, name=f"QB{c}")
              for c in range(NC)]                  # [Qhi; Qlo]
        for c in range(NC):
            nc.gpsimd.memset(KA[c][HDM:HDM + 1, :], 1.0)
        pv_sb = [singles.tile([P, 4, HDM + 1], f16, name=f"pv{c}")
                 for c in range(NC)]
        for c in range(NC):
            nc.gpsimd.memset(pv_sb[c][:, :, HDM:HDM + 1], 1.0)

        sb = ctx.enter_context(tc.tile_pool(name="sbwork", bufs=3))
        stat = ctx.enter_context(tc.tile_pool(name="stat", bufs=3))
        # PSUM bank map (8 banks): a/proj(2) pv/psv/f(1) b(2x2) o(1) tiny(1)
        # b tiles are [128,1024] spanning two banks: FOUR kt score blocks
        # per tile -> one [128,1024] exp per 4 kt. o holds BOTH group
        # tiles' accumulators in one bank (concurrent psum groups per
        # bank are per-address on HW).
        ppool = ctx.enter_context(tc.tile_pool(name="ppool", bufs=2, space="PSUM"))
        bpool = ctx.enter_context(tc.tile_pool(name="bpool", bufs=2, space="PSUM"))
        opool = ctx.enter_context(tc.tile_pool(name="opool", bufs=1, space="PSUM"))
        tiny = ctx.enter_context(tc.tile_pool(name="tiny", bufs=1, space="PSUM"))
        epool = ctx.enter_context(tc.tile_pool(name="epool", bufs=10))

        # shared o/psv bank: o accumulators for both group tiles at
        # om[:, 0:65] / om[:, 128:193]; psv occupies the back half as a
        # [128, 4, 64] view.
        om = opool.tile([P, 4 * P], f32, tag="om", name="om")
        om_r = om.rearrange("p (x u h) -> p x u h", x=2, u=4)

        # ---------- emission helpers (generator-style weaving) ----------

        def proj_chunk(c):
            """Generator: projection + extraction + pv for chunk c, with
            yields between matmul groups so leftover b work from the
            previous chunk can interleave instead of stalling on the exp
            cadence."""
            sl = slice(c * SC, (c + 1) * SC)
            ps = ppool.tile([P, SC], f32, tag="a", name="ps")
            # 3-term compensated f16 projection; ih-only terms first so the
            # il DMA of this slice is off the critical path. For chunk 0
            # the wh terms all come first (wh and the per-d ih sub-slices
            # land before wl does).
            if c == 0:
                order = ([("h", d) for d in range(4)]
                         + [("l", d) for d in range(4)])
            else:
                order = [w_d for d in range(4) for w_d in (("h", d), ("l", d))]
            for i, (w, d) in enumerate(order):
                w_sb = wh_sb if w == "h" else wl_sb
                nc.tensor.matmul(
                    ps, lhsT=w_sb[:, d, :], rhs=ih_sb[:, d, sl],
                    start=(i == 0), stop=False)
                if i % 2 == 1:
                    yield
            for d in range(4):
                nc.tensor.matmul(
                    ps, lhsT=wh_sb[:, d, :], rhs=il_sb[:, d, sl],
                    start=False, stop=(d == 3))
                yield
            # K extraction: hi (ACT) -> lo = ps - hi (DVE) -> hi rep (DVE)
            nc.scalar.copy(out=KA[c][0:HDM, :], in_=ps[HDM:P, :])
            nc.vector.tensor_tensor(
                out=KB[c][0:HDM, :], in0=ps[HDM:P, :], in1=KA[c][0:HDM, :],
                op=mybir.AluOpType.subtract)
            nc.vector.tensor_copy(KB[c][HDM:P, :], KA[c][0:HDM, :])
            # Q extraction for own tiles 2c, 2c+1 (even psum block positions)
            for u in range(2):
                qsl = slice(u * P, (u + 1) * P)
                psl = slice(2 * u * P, (2 * u + 1) * P)
                nc.scalar.copy(out=QA[c][0:HDM, qsl], in_=ps[0:HDM, psl])
                nc.vector.tensor_tensor(
                    out=QB[c][HDM:P, qsl], in0=ps[0:HDM, psl],
                    in1=QA[c][0:HDM, qsl], op=mybir.AluOpType.subtract)
                nc.vector.tensor_copy(QB[c][0:HDM, qsl], QA[c][0:HDM, qsl])
            # pv projection: 4 kt batched into the psv region of the shared
            # o/psv psum bank
            psv_r = om_r[:, 1, :, :]
            for u in range(4):
                kt = 4 * c + u
                for d in range(4):
                    nc.tensor.matmul(
                        psv_r[:, u, :],
                        lhsT=ih_sb[:, d, kt * P:(kt + 1) * P],
                        rhs=v1_sb[:, d, :], start=(d == 0), stop=(d == 3),
                        skip_group_check=True)
                yield
            nc.scalar.copy(out=pv_sb[c][:, :, 0:HDM], in_=psv_r)

        def pass_a_steps(t):
            """Generator: one yield per PE instruction block of pass A for
            tile t, so the caller can interleave b-score work between psum
            uses (reduce on DVE is ~3x slower than the feeding matmul)."""
            fc = t // 2
            dw = 256 if t % 2 == 0 else 512
            off = dw - 256
            lhsT = QA[fc][0:HDM, (t % 2) * P:(t % 2 + 1) * P]
            mxp = stat.tile([P, 10], f32, tag="mxp", name="mxp")
            for c in range(fc):
                aps = ppool.tile([P, SC], f32, tag="a", name="aps")
                nc.tensor.matmul(
                    aps, lhsT=lhsT, rhs=KA[c][0:HDM, :],
                    start=True, stop=True)
                nc.vector.reduce_max(
                    out=mxp[:, c:c + 1], in_=aps, axis=mybir.AxisListType.X)
                yield
            dps = ppool.tile([P, SC], f32, tag="a", name="dps")
            nc.tensor.matmul(
                dps[:, 0:dw], lhsT=lhsT,
                rhs=KA[fc][0:HDM, 0:dw], start=True, stop=False)
            nc.tensor.matmul(
                dps[:, off:off + P], lhsT=id16, rhs=maskd_sb[:, 2, :],
                start=False, stop=False)
            nc.tensor.matmul(
                dps[:, off + P:off + 2 * P], lhsT=id16, rhs=maskd_sb[:, 1, :],
                start=False, stop=True)
            nc.vector.reduce_max(
                out=mxp[:, fc:fc + 1], in_=dps[:, 0:dw],
                axis=mybir.AxisListType.X)
            yield
            # final combine (sbuf->sbuf) + mneg on Pool; transpose on PE;
            # row write via ACT (psum->sbuf)
            m = stat.tile([P, 1], f32, tag="m", name="m")
            nc.vector.reduce_max(
                out=m, in_=mxp[:, 0:fc + 1], axis=mybir.AxisListType.X)
            mneg = stat.tile([P, 1], f16, tag="mneg", name="mneg")
            nc.gpsimd.tensor_scalar(
                out=mneg, in0=m, scalar1=-1.0, scalar2=-32.0,
                op0=mybir.AluOpType.mult, op1=mybir.AluOpType.add)
            mt = tiny.tile([1, P], f16, tag="tp", name="mt")
            nc.tensor.transpose(mt, mneg, id16)
            # on DVE: lands right behind this tile's final reduce in the
            # FIFO, not behind the previous group's exp queue on ACT
            nc.vector.tensor_copy(
                QA[fc][HDM:HDM + 1, (t % 2) * P:(t % 2 + 1) * P], mt)
            yield

        def b_score_steps(tstart, ntiles, kt_hi):
            """Generator for group of `ntiles` tiles starting at `tstart`
            (q width ntiles*128): per kt two score matmuls + masks, kt
            blocks packed into [128,1024] two-bank psum tiles, one exp per
            psum tile. One yield per kt."""
            qw = ntiles * P
            kpt = 1024 // qw                   # kt blocks per psum tile
            qsl = slice((tstart % 2) * P, (tstart % 2) * P + qw)
            gq = tstart // 2
            es = []
            for ktq in range(0, kt_hi, kpt):
                nk = min(kpt, kt_hi - ktq)
                bps = bpool.tile([P, 1024], f32, tag="b", name="bps")
                for hh in range(nk):
                    kt = ktq + hh
                    kc = slice((kt % 4) * P, (kt % 4 + 1) * P)
                    o0 = hh * qw
                    in_band = kt >= 2 * tstart
                    nc.tensor.matmul(
                        bps[:, o0:o0 + qw], lhsT=KA[kt // 4][:, kc],
                        rhs=QA[gq][:, qsl],
                        start=True, stop=False, skip_group_check=True)
                    nc.tensor.matmul(
                        bps[:, o0:o0 + qw], lhsT=KB[kt // 4][:, kc],
                        rhs=QB[gq][:, qsl],
                        start=False, stop=not in_band, skip_group_check=True)
                    if in_band:                # diag tri / partner mask add
                        tl = kt // 2 - tstart
                        mi = kt % 2            # 0: tri mask, 1: partner mask
                        nc.tensor.matmul(
                            bps[:, o0 + tl * P:o0 + (tl + 1) * P], lhsT=id16,
                            rhs=maskd_sb[:, mi, :], start=False, stop=True,
                            skip_group_check=True)
                    yield
                e_sb = epool.tile([P, 1024], f16, tag="e", name="e_sb")
                nc.scalar.activation(
                    out=e_sb[:, 0:nk * qw], in_=bps[:, 0:nk * qw],
                    func=mybir.ActivationFunctionType.Exp, scale=0.125)
                es.append(e_sb)
            # O accumulation + finalize per tile; both tiles' accumulators
            # live in the shared om bank at 512B-aligned offsets.
            for tl in range(ntiles):
                t_abs = tstart + tl
                o_ps = om[:, tl * P:tl * P + HDM + 1]
                last = 2 * t_abs + 1
                for kt in range(0, last + 1):
                    e_sb = es[kt // kpt]
                    hh = kt % kpt
                    nc.tensor.matmul(
                        o_ps, lhsT=e_sb[:, hh * qw + tl * P:
                                        hh * qw + (tl + 1) * P],
                        rhs=pv_sb[kt // 4][:, kt % 4, :],
                        start=(kt == 0), stop=(kt == last),
                        skip_group_check=True)
                # finalize tile t_abs
                linv = stat.tile([P, 1], f32, tag="linv", name="linv")
                nc.vector.reciprocal(linv, o_ps[:, HDM:HDM + 1])
                on_sb = sb.tile([P, HDM], f16, tag="on", name="on_sb")
                nc.scalar.activation(
                    out=on_sb, in_=o_ps[:, 0:HDM],
                    func=mybir.ActivationFunctionType.Copy, scale=linv)
                ot_ps = tiny.tile([HDM, P], f16, tag="tp", name="ot_ps")
                nc.tensor.transpose(ot_ps, on_sb, id16)
                ot_sb = sb.tile([HDM, P], f16, tag="otsb", name="ot_sb")
                nc.vector.tensor_copy(ot_sb, ot_ps)
                f_ps = bpool.tile([P, IDM], f32, tag="b", name="f_ps")
                nc.tensor.matmul(
                    f_ps, lhsT=ot_sb, rhs=v2_sb, start=True, stop=True)
                f_sb = sb.tile([P, IDM], f32, tag="fsb", name="f_sb")
                if tl == 0:
                    nc.vector.tensor_copy(f_sb, f_ps)
                else:
                    nc.scalar.copy(out=f_sb, in_=f_ps)
                nc.sync.dma_start(out=out[t_abs], in_=f_sb)
                yield

        def drain(gen):
            if gen is not None:
                for _ in gen:
                    pass

        # ---- main loop: chunk c does proj/extraction/pv + pass A of tiles
        # 2c, 2c+1 interleaved with pair-group (c-1) score/exp/O work,
        # paced evenly across the pass-A psum slots.
        from itertools import chain

        def weave(slots, nslots, bgen, nitems):
            rate = nitems / nslots if nslots else 0.0
            frac = 0.0
            for _ in slots:
                frac += rate
                while frac >= 1.0 and bgen is not None:
                    next(bgen, None)
                    frac -= 1.0
            drain(bgen)

        bgen = None
        NCk = LN // SC
        for c in range(NCk - 1):
            drain(proj_chunk(c))
            slots = chain(pass_a_steps(2 * c), pass_a_steps(2 * c + 1))
            weave(slots, 2 * (c + 2), bgen,
                  (4 * (c - 1) + 6) if c else 0)
            bgen = b_score_steps(2 * c, 2, 4 * c + 4)
        # last chunk: singleton groups so tile 15's work alone forms the
        # tail and tile 14's work fills the final reduce phase
        c = NCk - 1
        drain(proj_chunk(c))
        weave(pass_a_steps(2 * c), c + 2, bgen, 4 * (c - 1) + 6)
        weave(pass_a_steps(2 * c + 1), c + 2,
              b_score_steps(2 * c, 1, 4 * c + 2), 4 * c + 3)
        drain(b_score_steps(2 * c + 1, 1, 4 * c + 4))

    nc.finalize()
    return nc


def make_core_inputs_v4(inputs):
    i = np.asarray(inputs["i"], dtype=np.float32)
    q = np.asarray(inputs["q"], dtype=np.float32)
    k = np.asarray(inputs["k"], dtype=np.float32)
    v1 = np.asarray(inputs["v1"], dtype=np.float32)
    v2 = np.asarray(inputs["v2"], dtype=np.float32)
    v2h = np.ascontiguousarray(v2.astype(np.float16))
    v1b = np.ascontiguousarray(v1.astype(np.float16))
    wqk = np.concatenate([q, k], axis=1)
    wh = wqk.astype(np.float16)
    wl = (wqk - wh.astype(np.float32)).astype(np.float16)
    whl = np.ascontiguousarray(np.stack([wh, wl]))
    iota = np.arange(P, dtype=np.float32)
    # tri mask for S^T diag block: valid iff k_local <= q_local
    tri = np.where(iota[:, None] <= iota[None, :], 0.0, MASK_NEG).astype(np.float16)
    # row-major tri for pass A: valid iff k_local <= q_local (q on partitions)
    tri_r = np.where(iota[None, :] <= iota[:, None], 0.0, MASK_NEG).astype(np.float16)
    in_maps = []
    for core in range(8):
        b, h = core // 2, core % 2
        perm_blocks = []
        for j in range(NT):
            perm_blocks += [2 * j + h, 2 * j + 1 - h]
        cols = np.concatenate(
            [np.arange(P * g, P * g + P) for g in perm_blocks])
        iTp = np.ascontiguousarray(i[b].T[:, cols])      # [512, 4096]
        maskd = np.stack([
            tri,
            np.full((P, P), 0.0 if h == 1 else MASK_NEG, dtype=np.float16),
            tri_r,
        ]).astype(np.float16)
        ihp = iTp.astype(np.float16)
        ilp = (iTp - ihp.astype(np.float32)).astype(np.float16)
        in_maps.append({
            "ihl": np.ascontiguousarray(np.stack([ihp, ilp])),
            "whl": whl,
            "v1s": v1b, "v2h": v2h,
            "maskd": np.ascontiguousarray(maskd),
        })
    return in_maps


_NC_CACHE = {}


def run_v3(inputs, **spmd_kwargs):
    if "v4" not in _NC_CACHE:
        _NC_CACHE["v4"] = build_nc_v4()
    nc = _NC_CACHE["v4"]
    in_maps = make_core_inputs_v4(inputs)
    res = run_bass_kernel_spmd(nc, in_maps, core_ids=list(range(8)), **spmd_kwargs)
    full = np.zeros((B, LN, IDM), dtype=np.float32)
    for core in range(8):
        b, h = core // 2, core % 2
        o = res.results[core]["out"]                 # [16, 128, 512] f32
        for t in range(NT):
            g = 2 * t + h
            full[b, g * P:(g + 1) * P] = o[t]
    return full, res


def kernel(**inputs):
    full, _ = run_v3(inputs)
    return full
